# revision 12
# baseline (speedup 1.0000x reference)
"""ABMIL aggregator (gated attention MIL) on 8 TRN2 NeuronCores.

Sharding: segment-aligned split -- core k owns slides 2k and 2k+1 (whole
slides stay on one core), so the per-segment softmax and weighted sums are
fully core-local and no collectives are needed. The small weights are
replicated; the host gathers the per-core outputs.

Math (per core, n patches padded to a multiple of 128):
  V|U  = x @ [Wv|Wu] + [bv|bu]          (PE, bf16, K=1024 in 8 chunks,
                                          bias added via a K=1 matmul)
  t1   = tanh(V), t2 = tanh(U/2)         (ACT; sigmoid(x)=(1+tanh(x/2))/2
                                          keeps tanh+exp in one table set)
  A    = sum_h (w_h/2) * t1 * (1 + t2)   (two fused DVE tensor_tensor_reduce)
  u    = exp(A)                          (no max-subtraction: |A| <~ 1)
  c    = u * sel                         (sel = host one-hot of local slide)
  W_j += c^T @ x ; S_j += sum(c)         (PE, persistent PSUM accumulation)
  out: slides = W/S, attn = u * (sel0/S0 + sel1/S1)
"""

import os

import numpy as np
import ml_dtypes

import concourse.bass as bass
import concourse.mybir as mybir
import concourse.tile as tile
from concourse.bass_utils import run_bass_kernel_spmd
from concourse.vector_clock import ScopedClock

N_CORES = 8
D = 1024
H = 256
B = 16
P = 128

BF16 = mybir.dt.bfloat16
F32 = mybir.dt.float32
FP8 = mybir.dt.float8e4
USE_FP8 = bool(int(os.environ.get("ABMIL_FP8", "1")))

# Results of the most recent run (exec_time_ns etc.) for the test harness.
LAST_RESULT = None


def _patch_tile_drain():
    """The pinned walrus rejects a Drain carrying >1 sem wait ("Too many sync
    wait commands"). Spread the end-of-kernel waits over one NOP each."""

    def _drain_and_barrier(self, tick_clock, wait_clock):
        probe = self.nc.sync.nop(nofuse=True)
        wait_clock.add_sem_waits(
            probe.ins, ScopedClock({None: tick_clock.global_clock})
        )
        si = probe.ins.sync_info
        if si is not None and si.on_wait and len(si.on_wait) > 1:
            waits = list(si.on_wait)
            probe.ins.sync_info = mybir.SyncInfo(
                on_wait=[waits[0]], on_update=list(si.on_update or [])
            )
            for w in waits[1:]:
                n = self.nc.sync.nop(nofuse=True)
                n.ins.sync_info = mybir.SyncInfo(on_wait=[w], on_update=[])
        self.nc.sync.drain()
        self.nc.all_engine_barrier()
        popped = self.nc._tile_sem_poison_stack.pop()
        assert popped is self._sem_poison
        self.nc.clear_and_free_semaphores(list(self.sems.allocated().values()))
        self.nc.all_engine_barrier()

    tile.TileContext._drain_and_barrier = _drain_and_barrier


_patch_tile_drain()


def _split_multi_waits(nc: bass.Bass):
    """This walrus supports 1 sem wait per instruction (2 on EventSemaphore).
    Hoist extra waits onto preceding same-engine NOPs."""
    uid = 0
    for f in nc.m.functions:
        for blk in f.blocks:
            insts = blk.instructions
            out = []
            changed = False
            for inst in insts:
                si = inst.sync_info
                ow = list((si.on_wait if si else None) or [])
                cap = 2 if isinstance(inst, mybir.InstEventSemaphore) else 1
                if len(ow) > cap:
                    changed = True
                    for w in ow[:-cap]:
                        nop = mybir.InstNoOp(
                            name=f"{inst.name}-hw{uid}", ins=[], outs=[]
                        )
                        uid += 1
                        nop.engine = inst.engine
                        nop.sync_info = mybir.SyncInfo(on_wait=[w], on_update=[])
                        out.append(nop)
                    inst.sync_info = mybir.SyncInfo(
                        on_wait=ow[-cap:], on_update=list(si.on_update or [])
                    )
                out.append(inst)
            if changed:
                blk.instructions = out


def _build_nc(ntiles: int, fp8: bool = USE_FP8) -> bass.Bass:
    n_pad = ntiles * P
    nc = bass.Bass()

    xn_d = nc.dram_tensor("xn", [n_pad, D], BF16, kind="ExternalInput")
    if fp8:
        # x^T in DoubleRow interleave: xt[t, ki, c, ko, n] = x[t*P+n, c*256+ko*128+ki]
        xt_d = nc.dram_tensor("xt", [ntiles, P, 4, 2, P], FP8, kind="ExternalInput")
        wvwu_d = nc.dram_tensor("wvwu", [P, 4, 2, 2 * H], FP8, kind="ExternalInput")
    else:
        xt_d = nc.dram_tensor("xt", [ntiles, P, 8, P], BF16, kind="ExternalInput")
        wvwu_d = nc.dram_tensor("wvwu", [8, P, 2 * H], BF16, kind="ExternalInput")
    sel_d = nc.dram_tensor("sel", [P, ntiles, 2], F32, kind="ExternalInput")
    brow_d = nc.dram_tensor("brow", [1, 2 * H], BF16, kind="ExternalInput")
    wtile_d = nc.dram_tensor("wtile", [P, H], BF16, kind="ExternalInput")
    id2_d = nc.dram_tensor("id2", [2, 2], F32, kind="ExternalInput")

    slides_d = nc.dram_tensor("slides", [2, D], F32, kind="ExternalOutput")
    attn_d = nc.dram_tensor("attn", [P, ntiles], F32, kind="ExternalOutput")

    mult = mybir.AluOpType.mult
    add = mybir.AluOpType.add
    Tanh = mybir.ActivationFunctionType.Tanh
    Exp = mybir.ActivationFunctionType.Exp

    with tile.TileContext(nc) as tc:
        with (
            tc.tile_pool(name="consts", bufs=1) as consts,
            tc.tile_pool(name="xn_pool", bufs=4) as xn_pool,
            tc.tile_pool(name="xt_pool", bufs=4) as xt_pool,
            tc.tile_pool(name="act_pool", bufs=3) as act_pool,
            tc.tile_pool(name="small", bufs=4) as small,
            tc.tile_pool(name="vu_psum", bufs=2, space="PSUM") as vu_psum,
            tc.tile_pool(name="acc_psum", bufs=1, space="PSUM") as acc_psum,
        ):
            if fp8:
                wvwu_s = consts.tile([P, 4, 2, 2 * H], FP8, name="wvwu_s")
                nc.sync.dma_start(out=wvwu_s, in_=wvwu_d.ap())
            else:
                wvwu_s = consts.tile([P, 8, 2 * H], BF16, name="wvwu_s")
                nc.sync.dma_start(
                    out=wvwu_s, in_=wvwu_d.ap().rearrange("c p h -> p c h")
                )
            brow_s = consts.tile([1, 2 * H], BF16, name="brow_s")
            nc.sync.dma_start(out=brow_s, in_=brow_d.ap())
            wtile_s = consts.tile([P, H], BF16, name="wtile_s")
            nc.sync.dma_start(out=wtile_s, in_=wtile_d.ap())
            sel_s = consts.tile([P, ntiles, 2], F32, name="sel_s")
            nc.sync.dma_start(out=sel_s, in_=sel_d.ap())
            id2_s = consts.tile([2, 2], F32, name="id2_s")
            nc.sync.dma_start(out=id2_s, in_=id2_d.ap())

            ones_row = consts.tile([1, P], BF16, name="ones_row")
            nc.vector.memset(ones_row, 1.0)
            ones_col = consts.tile([P, 1], BF16, name="ones_col")
            nc.vector.memset(ones_col, 1.0)
            ones_row_f = consts.tile([1, P], F32, name="ones_row_f")
            nc.vector.memset(ones_row_f, 1.0)

            ubuf = consts.tile([P, ntiles], F32, name="ubuf")

            w_ps0 = acc_psum.tile([2, 512], F32, name="w_ps0")
            w_ps1 = acc_psum.tile([2, 512], F32, name="w_ps1")
            s_ps = acc_psum.tile([2, 1], F32, name="s_ps")

            for t in range(ntiles):
                first = t == 0
                last = t == ntiles - 1

                xn_t = xn_pool.tile([P, D], BF16, name="xn_t")
                nc.sync.dma_start(out=xn_t, in_=xn_d.ap()[t * P : (t + 1) * P, :])
                if fp8:
                    xt_t = xt_pool.tile([P, 4, 2, P], FP8, name="xt_t")
                else:
                    xt_t = xt_pool.tile([P, 8, P], BF16, name="xt_t")
                nc.sync.dma_start(out=xt_t, in_=xt_d.ap()[t])

                vu = vu_psum.tile([P, 2 * H], F32, name="vu", tag="vu")
                # PSUM init with broadcast bias row, then accumulate x @ W.
                nc.tensor.matmul(vu, ones_row, brow_s, start=True, stop=False)
                if fp8:
                    for c in range(4):
                        nc.tensor.matmul(
                            vu,
                            xt_t[:, c, :, :],
                            wvwu_s[:, c, :, :],
                            start=False,
                            stop=(c == 3),
                            perf_mode=mybir.MatmulPerfMode.DoubleRow,
                        )
                else:
                    for c in range(8):
                        nc.tensor.matmul(
                            vu,
                            xt_t[:, c, :],
                            wvwu_s[:, c, :],
                            start=False,
                            stop=(c == 7),
                        )

                t1 = act_pool.tile([P, H], BF16, name="t1")
                nc.scalar.activation(t1, vu[:, 0:H], Tanh)
                t2 = act_pool.tile([P, H], BF16, name="t2")
                nc.scalar.activation(t2, vu[:, H : 2 * H], Tanh, scale=0.5)

                # A = sum_h (w_h/2) * t1 * (1 + t2)
                g = act_pool.tile([P, H], BF16, name="g")
                nc.vector.tensor_mul(g, t1, t2)
                g2 = act_pool.tile([P, H], BF16, name="g2")
                nc.vector.tensor_add(g2, t1, g)
                gw = act_pool.tile([P, H], BF16, name="gw")
                nc.vector.tensor_mul(gw, g2, wtile_s)
                a_log = small.tile([P, 1], F32, name="a_log")
                nc.vector.reduce_sum(a_log, gw, axis=mybir.AxisListType.X)

                nc.scalar.activation(ubuf[:, t : t + 1], a_log, Exp)

                c_t = small.tile([P, 2], BF16, name="c_t")
                nc.vector.tensor_scalar_mul(c_t, sel_s[:, t, :], ubuf[:, t : t + 1])

                nc.tensor.matmul(
                    w_ps0, c_t, xn_t[:, 0:512],
                    start=first, stop=last, skip_group_check=True,
                )
                nc.tensor.matmul(
                    w_ps1, c_t, xn_t[:, 512:1024],
                    start=first, stop=last, skip_group_check=True,
                )
                nc.tensor.matmul(
                    s_ps, c_t, ones_col,
                    start=first, stop=last, skip_group_check=True,
                )

            # ---- epilogue ----
            invs = small.tile([2, 1], F32, name="invs")
            nc.vector.reciprocal(invs, s_ps)

            slides_s = consts.tile([2, D], F32, name="slides_s")
            nc.vector.tensor_scalar_mul(slides_s[:, 0:512], w_ps0, invs)
            nc.vector.tensor_scalar_mul(slides_s[:, 512:1024], w_ps1, invs)
            nc.sync.dma_start(out=slides_d.ap(), in_=slides_s)

            # Broadcast 1/S to all 128 partitions: [2,1] -T-> [1,2] -ones-> [128,2]
            invt_ps = vu_psum.tile([1, 2], F32, name="invt_ps", tag="vu")
            nc.tensor.matmul(invt_ps, invs, id2_s, start=True, stop=True)
            invt_s = small.tile([1, 2], F32, name="invt_s")
            nc.vector.tensor_copy(invt_s, invt_ps)
            bc_ps = vu_psum.tile([P, 2], F32, name="bc_ps", tag="vu")
            nc.tensor.matmul(bc_ps, ones_row_f, invt_s, start=True, stop=True)
            invb = small.tile([P, 2], F32, name="invb")
            nc.vector.tensor_copy(invb, bc_ps)

            ra = consts.tile([P, ntiles], F32, name="ra")
            nc.vector.tensor_scalar_mul(ra, sel_s[:, :, 0], invb[:, 0:1])
            rb = consts.tile([P, ntiles], F32, name="rb")
            nc.vector.tensor_scalar_mul(rb, sel_s[:, :, 1], invb[:, 1:2])
            rs = consts.tile([P, ntiles], F32, name="rs")
            nc.vector.tensor_add(rs, ra, rb)
            attn_s = consts.tile([P, ntiles], F32, name="attn_s")
            nc.vector.tensor_mul(attn_s, ubuf, rs)
            nc.sync.dma_start(out=attn_d.ap(), in_=attn_s)

    _split_multi_waits(nc)
    return nc


def prepare(inputs):
    """Build the Bass graph and per-core input maps from full inputs."""
    x = np.ascontiguousarray(np.asarray(inputs["x"], dtype=np.float32))
    batch = np.asarray(inputs["batch"]).astype(np.int64)
    Wv = np.asarray(inputs["Wv"], dtype=np.float32)
    bv = np.asarray(inputs["bv"], dtype=np.float32)
    Wu = np.asarray(inputs["Wu"], dtype=np.float32)
    bu = np.asarray(inputs["bu"], dtype=np.float32)
    w = np.asarray(inputs["w"], dtype=np.float32)

    n_total = x.shape[0]
    assert x.shape[1] == D and Wv.shape == (D, H) and w.shape == (H, 1)

    bounds = np.searchsorted(batch, np.arange(B + 1))
    sizes = [int(bounds[2 * k + 2] - bounds[2 * k]) for k in range(N_CORES)]
    ntiles = max(1, -(-max(sizes) // P))
    n_pad = ntiles * P

    nc = _build_nc(ntiles)

    fp8np = mybir.dt.np(FP8)
    # Replicated small tensors.
    w_cat = np.concatenate([Wv, Wu], axis=1)  # (D, 512)
    if USE_FP8:
        # wvwu[ki, c, ko, h] = w_cat[c*256 + ko*128 + ki, h]
        wvwu = np.ascontiguousarray(
            w_cat.reshape(4, 2, P, 2 * H).transpose(2, 0, 1, 3)
        ).astype(fp8np)
    else:
        wvwu = w_cat.reshape(8, P, 2 * H).astype(ml_dtypes.bfloat16)
    brow = np.concatenate([bv, bu]).reshape(1, 2 * H).astype(ml_dtypes.bfloat16)
    wtile = np.tile((0.5 * w.reshape(1, H)).astype(ml_dtypes.bfloat16), (P, 1))
    id2 = np.eye(2, dtype=np.float32)

    in_maps = []
    for k in range(N_CORES):
        lo, hi = int(bounds[2 * k]), int(bounds[2 * k + 2])
        nk = hi - lo
        xb = np.zeros((n_pad, D), dtype=ml_dtypes.bfloat16)
        xb[:nk] = x[lo:hi]
        if USE_FP8:
            # xt[t, ki, c, ko, n] = x[t*128 + n, c*256 + ko*128 + ki]
            x8 = np.zeros((n_pad, D), dtype=fp8np)
            x8[:nk] = x[lo:hi]
            xt = np.ascontiguousarray(
                x8.reshape(ntiles, P, 4, 2, P).transpose(0, 4, 2, 3, 1)
            )
        else:
            # xt[t, p, c, n] = x[t*128 + n, c*128 + p]
            xt = np.ascontiguousarray(
                xb.reshape(ntiles, P, 8, P).transpose(0, 3, 2, 1)
            )
        loc = np.full(n_pad, -1, dtype=np.int64)
        loc[:nk] = batch[lo:hi] - 2 * k
        sel = np.stack([(loc == 0), (loc == 1)], axis=-1).astype(np.float32)
        sel = np.ascontiguousarray(sel.reshape(ntiles, P, 2).transpose(1, 0, 2))
        in_maps.append(
            {
                "xn": xb,
                "xt": xt,
                "sel": sel,
                "wvwu": wvwu,
                "brow": brow,
                "wtile": wtile,
                "id2": id2,
            }
        )

    return nc, in_maps, bounds, n_total


def assemble(results, bounds, n_total):
    slide_features = np.concatenate(
        [results[k]["slides"] for k in range(N_CORES)], axis=0
    ).astype(np.float32)
    attn = np.empty((n_total, 1), dtype=np.float32)
    for k in range(N_CORES):
        lo, hi = int(bounds[2 * k]), int(bounds[2 * k + 2])
        nk = hi - lo
        a = results[k]["attn"]  # [P, ntiles]
        attn[lo:hi, 0] = np.ascontiguousarray(a.T).reshape(-1)[:nk]
    return slide_features, attn


def kernel(**inputs):
    global LAST_RESULT
    nc, in_maps, bounds, n_total = prepare(inputs)
    trace = bool(int(os.environ.get("ABMIL_TRACE", "0")))
    LAST_RESULT = run_bass_kernel_spmd(
        nc, in_maps, core_ids=list(range(N_CORES)), trace=trace
    )
    return assemble(LAST_RESULT.results, bounds, n_total)


# revision 14
# speedup vs baseline: 1.9254x; 1.9254x over previous
"""ABMIL aggregator (gated attention MIL) on 8 TRN2 NeuronCores.

Sharding: segment-aligned split -- core k owns slides 2k and 2k+1 (whole
slides stay on one core), so the per-segment softmax and weighted sums are
fully core-local and no collectives are needed. The small weights are
replicated; the host gathers the per-core outputs.

Math (per core, n patches padded to a multiple of 128):
  V|U  = x @ [Wv|Wu] + [bv|bu]          (PE, bf16, K=1024 in 8 chunks,
                                          bias added via a K=1 matmul)
  t1   = tanh(V), t2 = tanh(U/2)         (ACT; sigmoid(x)=(1+tanh(x/2))/2
                                          keeps tanh+exp in one table set)
  A    = sum_h (w_h/2) * t1 * (1 + t2)   (two fused DVE tensor_tensor_reduce)
  u    = exp(A)                          (no max-subtraction: |A| <~ 1)
  c    = u * sel                         (sel = host one-hot of local slide)
  W_j += c^T @ x ; S_j += sum(c)         (PE, persistent PSUM accumulation)
  out: slides = W/S, attn = u * (sel0/S0 + sel1/S1)
"""

import os

import numpy as np
import ml_dtypes

import concourse.bass as bass
import concourse.mybir as mybir
import concourse.tile as tile
from concourse.bass_utils import run_bass_kernel_spmd
from concourse.vector_clock import ScopedClock

N_CORES = 8
D = 1024
H = 256
B = 16
P = 128

BF16 = mybir.dt.bfloat16
F32 = mybir.dt.float32
FP8 = mybir.dt.float8e4
USE_FP8 = bool(int(os.environ.get("ABMIL_FP8", "1")))

# Results of the most recent run (exec_time_ns etc.) for the test harness.
LAST_RESULT = None


def _patch_tile_drain():
    """The pinned walrus rejects a Drain carrying >1 sem wait ("Too many sync
    wait commands"). Spread the end-of-kernel waits over one NOP each."""

    def _drain_and_barrier(self, tick_clock, wait_clock):
        probe = self.nc.sync.nop(nofuse=True)
        wait_clock.add_sem_waits(
            probe.ins, ScopedClock({None: tick_clock.global_clock})
        )
        si = probe.ins.sync_info
        if si is not None and si.on_wait and len(si.on_wait) > 1:
            waits = list(si.on_wait)
            probe.ins.sync_info = mybir.SyncInfo(
                on_wait=[waits[0]], on_update=list(si.on_update or [])
            )
            for w in waits[1:]:
                n = self.nc.sync.nop(nofuse=True)
                n.ins.sync_info = mybir.SyncInfo(on_wait=[w], on_update=[])
        self.nc.sync.drain()
        self.nc.all_engine_barrier()
        popped = self.nc._tile_sem_poison_stack.pop()
        assert popped is self._sem_poison
        self.nc.clear_and_free_semaphores(list(self.sems.allocated().values()))
        self.nc.all_engine_barrier()

    tile.TileContext._drain_and_barrier = _drain_and_barrier


_patch_tile_drain()


def _split_multi_waits(nc: bass.Bass):
    """This walrus supports 1 sem wait per instruction (2 on EventSemaphore).
    Hoist extra waits onto preceding same-engine NOPs."""
    uid = 0
    for f in nc.m.functions:
        for blk in f.blocks:
            insts = blk.instructions
            out = []
            changed = False
            for inst in insts:
                si = inst.sync_info
                ow = list((si.on_wait if si else None) or [])
                cap = 2 if isinstance(inst, mybir.InstEventSemaphore) else 1
                if len(ow) > cap:
                    changed = True
                    for w in ow[:-cap]:
                        nop = mybir.InstNoOp(
                            name=f"{inst.name}-hw{uid}", ins=[], outs=[]
                        )
                        uid += 1
                        nop.engine = inst.engine
                        nop.sync_info = mybir.SyncInfo(on_wait=[w], on_update=[])
                        out.append(nop)
                    inst.sync_info = mybir.SyncInfo(
                        on_wait=ow[-cap:], on_update=list(si.on_update or [])
                    )
                out.append(inst)
            if changed:
                blk.instructions = out


def _build_nc(ntiles: int, fp8: bool = USE_FP8, reps: int = 1) -> bass.Bass:
    n_pad = ntiles * P
    nc = bass.Bass()

    xn_d = nc.dram_tensor("xn", [n_pad, D], BF16, kind="ExternalInput")
    if fp8:
        # x^T in DoubleRow interleave: xt[t, ki, c, ko, n] = x[t*P+n, c*256+ko*128+ki]
        xt_d = nc.dram_tensor("xt", [ntiles, P, 4, 2, P], FP8, kind="ExternalInput")
        wvwu_d = nc.dram_tensor("wvwu", [P, 4, 2, 2 * H], FP8, kind="ExternalInput")
    else:
        xt_d = nc.dram_tensor("xt", [ntiles, P, 8, P], BF16, kind="ExternalInput")
        wvwu_d = nc.dram_tensor("wvwu", [8, P, 2 * H], BF16, kind="ExternalInput")
    sel_d = nc.dram_tensor("sel", [P, ntiles, 2], F32, kind="ExternalInput")
    brow_d = nc.dram_tensor("brow", [1, 2 * H], BF16, kind="ExternalInput")
    wtile_d = nc.dram_tensor("wtile", [P, H], BF16, kind="ExternalInput")
    id2_d = nc.dram_tensor("id2", [2, 2], F32, kind="ExternalInput")

    slides_d = nc.dram_tensor("slides", [2, D], F32, kind="ExternalOutput")
    attn_d = nc.dram_tensor("attn", [P, ntiles], F32, kind="ExternalOutput")

    mult = mybir.AluOpType.mult
    add = mybir.AluOpType.add
    Tanh = mybir.ActivationFunctionType.Tanh
    Exp = mybir.ActivationFunctionType.Exp

    with tile.TileContext(nc) as tc:
        with (
            tc.tile_pool(name="consts", bufs=1) as consts,
            tc.tile_pool(name="xn_pool", bufs=4) as xn_pool,
            tc.tile_pool(name="xt_pool", bufs=4) as xt_pool,
            tc.tile_pool(name="act_pool", bufs=3) as act_pool,
            tc.tile_pool(name="small", bufs=4) as small,
            tc.tile_pool(name="vu_psum", bufs=2, space="PSUM") as vu_psum,
            tc.tile_pool(name="acc_psum", bufs=1, space="PSUM") as acc_psum,
        ):
            if fp8:
                wvwu_s = consts.tile([P, 4, 2, 2 * H], FP8, name="wvwu_s")
                nc.sync.dma_start(out=wvwu_s, in_=wvwu_d.ap())
            else:
                wvwu_s = consts.tile([P, 8, 2 * H], BF16, name="wvwu_s")
                nc.sync.dma_start(
                    out=wvwu_s, in_=wvwu_d.ap().rearrange("c p h -> p c h")
                )
            brow_s = consts.tile([1, 2 * H], BF16, name="brow_s")
            nc.sync.dma_start(out=brow_s, in_=brow_d.ap())
            wtile_s = consts.tile([P, H], BF16, name="wtile_s")
            nc.sync.dma_start(out=wtile_s, in_=wtile_d.ap())
            sel_s = consts.tile([P, ntiles, 2], F32, name="sel_s")
            nc.sync.dma_start(out=sel_s, in_=sel_d.ap())
            id2_s = consts.tile([2, 2], F32, name="id2_s")
            nc.sync.dma_start(out=id2_s, in_=id2_d.ap())

            ones_row = consts.tile([1, P], BF16, name="ones_row")
            nc.vector.memset(ones_row, 1.0)
            ones_col = consts.tile([P, 1], BF16, name="ones_col")
            nc.vector.memset(ones_col, 1.0)
            ones_row_f = consts.tile([1, P], F32, name="ones_row_f")
            nc.vector.memset(ones_row_f, 1.0)

            ubuf = consts.tile([P, ntiles], F32, name="ubuf")

            w_ps0 = acc_psum.tile([2, 512], F32, name="w_ps0")
            w_ps1 = acc_psum.tile([2, 512], F32, name="w_ps1")
            s_ps = acc_psum.tile([2, 1], F32, name="s_ps")

            for rep in range(reps):
              for t in range(ntiles):
                first = rep == 0 and t == 0
                last = rep == reps - 1 and t == ntiles - 1

                xn_t = xn_pool.tile([P, D], BF16, name="xn_t")
                nc.sync.dma_start(out=xn_t, in_=xn_d.ap()[t * P : (t + 1) * P, :])
                if fp8:
                    xt_t = xt_pool.tile([P, 4, 2, P], FP8, name="xt_t")
                else:
                    xt_t = xt_pool.tile([P, 8, P], BF16, name="xt_t")
                nc.sync.dma_start(out=xt_t, in_=xt_d.ap()[t])

                vu = vu_psum.tile([P, 2 * H], F32, name="vu", tag="vu")
                # PSUM init with broadcast bias row, then accumulate x @ W.
                nc.tensor.matmul(vu, ones_row, brow_s, start=True, stop=False)
                if fp8:
                    for c in range(4):
                        nc.tensor.matmul(
                            vu,
                            xt_t[:, c, :, :],
                            wvwu_s[:, c, :, :],
                            start=False,
                            stop=(c == 3),
                            perf_mode=mybir.MatmulPerfMode.DoubleRow,
                        )
                else:
                    for c in range(8):
                        nc.tensor.matmul(
                            vu,
                            xt_t[:, c, :],
                            wvwu_s[:, c, :],
                            start=False,
                            stop=(c == 7),
                        )

                t1 = act_pool.tile([P, H], BF16, name="t1")
                nc.scalar.activation(t1, vu[:, 0:H], Tanh)
                t2 = act_pool.tile([P, H], BF16, name="t2")
                nc.scalar.activation(t2, vu[:, H : 2 * H], Tanh, scale=0.5)

                # A = sum_h (w_h/2) * t1 * (1 + t2)
                g = act_pool.tile([P, H], BF16, name="g")
                nc.vector.tensor_mul(g, t1, t2)
                g2 = act_pool.tile([P, H], BF16, name="g2")
                nc.vector.tensor_add(g2, t1, g)
                gw = act_pool.tile([P, H], BF16, name="gw")
                nc.vector.tensor_mul(gw, g2, wtile_s)
                a_log = small.tile([P, 1], F32, name="a_log")
                nc.vector.reduce_sum(a_log, gw, axis=mybir.AxisListType.X)

                nc.scalar.activation(ubuf[:, t : t + 1], a_log, Exp)

                c_t = small.tile([P, 2], BF16, name="c_t")
                nc.vector.tensor_scalar_mul(c_t, sel_s[:, t, :], ubuf[:, t : t + 1])

                nc.tensor.matmul(
                    w_ps0, c_t, xn_t[:, 0:512],
                    start=first, stop=last, skip_group_check=True,
                )
                nc.tensor.matmul(
                    w_ps1, c_t, xn_t[:, 512:1024],
                    start=first, stop=last, skip_group_check=True,
                )
                nc.tensor.matmul(
                    s_ps, c_t, ones_col,
                    start=first, stop=last, skip_group_check=True,
                )

            # ---- epilogue ----
            invs = small.tile([2, 1], F32, name="invs")
            nc.vector.reciprocal(invs, s_ps)

            slides_s = consts.tile([2, D], F32, name="slides_s")
            nc.vector.tensor_scalar_mul(slides_s[:, 0:512], w_ps0, invs)
            nc.vector.tensor_scalar_mul(slides_s[:, 512:1024], w_ps1, invs)
            nc.sync.dma_start(out=slides_d.ap(), in_=slides_s)

            # Broadcast 1/S to all 128 partitions: [2,1] -T-> [1,2] -ones-> [128,2]
            invt_ps = vu_psum.tile([1, 2], F32, name="invt_ps", tag="vu")
            nc.tensor.matmul(invt_ps, invs, id2_s, start=True, stop=True)
            invt_s = small.tile([1, 2], F32, name="invt_s")
            nc.vector.tensor_copy(invt_s, invt_ps)
            bc_ps = vu_psum.tile([P, 2], F32, name="bc_ps", tag="vu")
            nc.tensor.matmul(bc_ps, ones_row_f, invt_s, start=True, stop=True)
            invb = small.tile([P, 2], F32, name="invb")
            nc.vector.tensor_copy(invb, bc_ps)

            ra = consts.tile([P, ntiles], F32, name="ra")
            nc.vector.tensor_scalar_mul(ra, sel_s[:, :, 0], invb[:, 0:1])
            rb = consts.tile([P, ntiles], F32, name="rb")
            nc.vector.tensor_scalar_mul(rb, sel_s[:, :, 1], invb[:, 1:2])
            rs = consts.tile([P, ntiles], F32, name="rs")
            nc.vector.tensor_add(rs, ra, rb)
            attn_s = consts.tile([P, ntiles], F32, name="attn_s")
            nc.vector.tensor_mul(attn_s, ubuf, rs)
            nc.sync.dma_start(out=attn_d.ap(), in_=attn_s)

    _split_multi_waits(nc)
    return nc


def prepare(inputs):
    """Build the Bass graph and per-core input maps from full inputs."""
    x = np.ascontiguousarray(np.asarray(inputs["x"], dtype=np.float32))
    batch = np.asarray(inputs["batch"]).astype(np.int64)
    Wv = np.asarray(inputs["Wv"], dtype=np.float32)
    bv = np.asarray(inputs["bv"], dtype=np.float32)
    Wu = np.asarray(inputs["Wu"], dtype=np.float32)
    bu = np.asarray(inputs["bu"], dtype=np.float32)
    w = np.asarray(inputs["w"], dtype=np.float32)

    n_total = x.shape[0]
    assert x.shape[1] == D and Wv.shape == (D, H) and w.shape == (H, 1)

    bounds = np.searchsorted(batch, np.arange(B + 1))
    sizes = [int(bounds[2 * k + 2] - bounds[2 * k]) for k in range(N_CORES)]
    ntiles = max(1, -(-max(sizes) // P))
    n_pad = ntiles * P

    nc = _build_nc(ntiles)

    fp8np = mybir.dt.np(FP8)
    # Replicated small tensors.
    w_cat = np.concatenate([Wv, Wu], axis=1)  # (D, 512)
    if USE_FP8:
        # wvwu[ki, c, ko, h] = w_cat[c*256 + ko*128 + ki, h]
        wvwu = np.ascontiguousarray(
            w_cat.reshape(4, 2, P, 2 * H).transpose(2, 0, 1, 3)
        ).astype(fp8np)
    else:
        wvwu = w_cat.reshape(8, P, 2 * H).astype(ml_dtypes.bfloat16)
    brow = np.concatenate([bv, bu]).reshape(1, 2 * H).astype(ml_dtypes.bfloat16)
    wtile = np.tile((0.5 * w.reshape(1, H)).astype(ml_dtypes.bfloat16), (P, 1))
    id2 = np.eye(2, dtype=np.float32)

    in_maps = []
    for k in range(N_CORES):
        lo, hi = int(bounds[2 * k]), int(bounds[2 * k + 2])
        nk = hi - lo
        xb = np.zeros((n_pad, D), dtype=ml_dtypes.bfloat16)
        xb[:nk] = x[lo:hi]
        if USE_FP8:
            # xt[t, ki, c, ko, n] = x[t*128 + n, c*256 + ko*128 + ki]
            x8 = np.zeros((n_pad, D), dtype=fp8np)
            x8[:nk] = x[lo:hi]
            xt = np.ascontiguousarray(
                x8.reshape(ntiles, P, 4, 2, P).transpose(0, 4, 2, 3, 1)
            )
        else:
            # xt[t, p, c, n] = x[t*128 + n, c*128 + p]
            xt = np.ascontiguousarray(
                xb.reshape(ntiles, P, 8, P).transpose(0, 3, 2, 1)
            )
        loc = np.full(n_pad, -1, dtype=np.int64)
        loc[:nk] = batch[lo:hi] - 2 * k
        sel = np.stack([(loc == 0), (loc == 1)], axis=-1).astype(np.float32)
        sel = np.ascontiguousarray(sel.reshape(ntiles, P, 2).transpose(1, 0, 2))
        in_maps.append(
            {
                "xn": xb,
                "xt": xt,
                "sel": sel,
                "wvwu": wvwu,
                "brow": brow,
                "wtile": wtile,
                "id2": id2,
            }
        )

    return nc, in_maps, bounds, n_total


def assemble(results, bounds, n_total):
    slide_features = np.concatenate(
        [results[k]["slides"] for k in range(N_CORES)], axis=0
    ).astype(np.float32)
    attn = np.empty((n_total, 1), dtype=np.float32)
    for k in range(N_CORES):
        lo, hi = int(bounds[2 * k]), int(bounds[2 * k + 2])
        nk = hi - lo
        a = results[k]["attn"]  # [P, ntiles]
        attn[lo:hi, 0] = np.ascontiguousarray(a.T).reshape(-1)[:nk]
    return slide_features, attn


def kernel(**inputs):
    global LAST_RESULT
    nc, in_maps, bounds, n_total = prepare(inputs)
    trace = bool(int(os.environ.get("ABMIL_TRACE", "0")))
    LAST_RESULT = run_bass_kernel_spmd(
        nc, in_maps, core_ids=list(range(N_CORES)), trace=trace
    )
    return assemble(LAST_RESULT.results, bounds, n_total)


# revision 36
# speedup vs baseline: 2.7060x; 1.4054x over previous
"""ABMIL aggregator (gated attention MIL) on 8 TRN2 NeuronCores.

Sharding: segment-aligned split -- core k owns slides 2k and 2k+1 (whole
slides stay on one core), so the per-segment softmax and weighted sums are
fully core-local and no collectives are needed. The small weights are
replicated; the host gathers the per-core outputs.

Math (per core, n patches padded to a multiple of 128):
  V|U  = x @ [Wv|Wu] + [bv|bu]          (PE, bf16, K=1024 in 8 chunks,
                                          bias added via a K=1 matmul)
  t1   = tanh(V), t2 = tanh(U/2)         (ACT; sigmoid(x)=(1+tanh(x/2))/2
                                          keeps tanh+exp in one table set)
  A    = sum_h (w_h/2) * t1 * (1 + t2)   (two fused DVE tensor_tensor_reduce)
  u    = exp(A)                          (no max-subtraction: |A| <~ 1)
  c    = u * sel                         (sel = host one-hot of local slide)
  W_j += c^T @ x ; S_j += sum(c)         (PE, persistent PSUM accumulation)
  out: slides = W/S, attn = u * (sel0/S0 + sel1/S1)
"""

import os

import numpy as np
import ml_dtypes

import concourse.bass as bass
import concourse.mybir as mybir
import concourse.tile as tile
from concourse.bass_utils import run_bass_kernel_spmd
from concourse.vector_clock import ScopedClock

N_CORES = 8
D = 1024
H = 256
B = 16
P = 128

BF16 = mybir.dt.bfloat16
F32 = mybir.dt.float32
FP8 = mybir.dt.float8e4
# matmul mode for the V/U projection: bf16 | fp8dr | fp8sw
MM_MODE = os.environ.get("ABMIL_MM", "fp8sw")
USE_FP8 = MM_MODE in ("fp8dr", "fp8sw")
# fp8 pre-scales (keep W out of e4m3's subnormal range); undone in ACT scale
SW_SCALE = 32.0
SX_SCALE = 4.0

# Results of the most recent run (exec_time_ns etc.) for the test harness.
LAST_RESULT = None


def _patch_tile_drain():
    """The pinned walrus rejects a Drain carrying >1 sem wait ("Too many sync
    wait commands"). Spread the end-of-kernel waits over one NOP each."""

    def _drain_and_barrier(self, tick_clock, wait_clock):
        probe = self.nc.sync.nop(nofuse=True)
        wait_clock.add_sem_waits(
            probe.ins, ScopedClock({None: tick_clock.global_clock})
        )
        si = probe.ins.sync_info
        if si is not None and si.on_wait and len(si.on_wait) > 1:
            waits = list(si.on_wait)
            probe.ins.sync_info = mybir.SyncInfo(
                on_wait=[waits[0]], on_update=list(si.on_update or [])
            )
            for w in waits[1:]:
                n = self.nc.sync.nop(nofuse=True)
                n.ins.sync_info = mybir.SyncInfo(on_wait=[w], on_update=[])
        self.nc.sync.drain()
        self.nc.all_engine_barrier()
        popped = self.nc._tile_sem_poison_stack.pop()
        assert popped is self._sem_poison
        self.nc.clear_and_free_semaphores(list(self.sems.allocated().values()))
        self.nc.all_engine_barrier()

    tile.TileContext._drain_and_barrier = _drain_and_barrier


_patch_tile_drain()


def _split_multi_waits(nc: bass.Bass):
    """This walrus supports 1 sem wait per instruction (2 on EventSemaphore).
    Hoist extra waits onto preceding same-engine NOPs."""
    uid = 0
    for f in nc.m.functions:
        for blk in f.blocks:
            insts = blk.instructions
            out = []
            changed = False
            for inst in insts:
                si = inst.sync_info
                ow = list((si.on_wait if si else None) or [])
                cap = 2 if isinstance(inst, mybir.InstEventSemaphore) else 1
                if len(ow) > cap:
                    changed = True
                    for w in ow[:-cap]:
                        nop = mybir.InstNoOp(
                            name=f"{inst.name}-hw{uid}", ins=[], outs=[]
                        )
                        uid += 1
                        nop.engine = inst.engine
                        nop.sync_info = mybir.SyncInfo(on_wait=[w], on_update=[])
                        out.append(nop)
                    inst.sync_info = mybir.SyncInfo(
                        on_wait=ow[-cap:], on_update=list(si.on_update or [])
                    )
                out.append(inst)
            if changed:
                blk.instructions = out


def _build_nc(
    ntiles: int, fp8: bool = USE_FP8, reps: int = 1, mode: str = "full"
) -> bass.Bass:
    """mode: "full" | "dma" (loads only) | "compute" (loads hoisted out of loop)."""
    n_pad = ntiles * P
    nc = bass.Bass()

    xn_d = nc.dram_tensor("xn", [n_pad, D], BF16, kind="ExternalInput")
    if fp8:
        # x^T in DoubleRow interleave: xt[t, ki, c, ko, n] = x[t*P+n, c*256+ko*128+ki]
        xt_d = nc.dram_tensor("xt", [ntiles, P, 4, 2, P], FP8, kind="ExternalInput")
        wvwu_d = nc.dram_tensor("wvwu", [P, 4, 2, 2 * H], FP8, kind="ExternalInput")
    else:
        xt_d = nc.dram_tensor("xt", [ntiles, P, 8, P], BF16, kind="ExternalInput")
        wvwu_d = nc.dram_tensor("wvwu", [8, P, 2 * H], BF16, kind="ExternalInput")
    sel_d = nc.dram_tensor("sel", [P, ntiles, 2], F32, kind="ExternalInput")
    brow_d = nc.dram_tensor("brow", [1, 2 * H], BF16, kind="ExternalInput")
    wtile_d = nc.dram_tensor("wtile", [P, H], BF16, kind="ExternalInput")
    id2_d = nc.dram_tensor("id2", [2, 2], F32, kind="ExternalInput")

    slides_d = nc.dram_tensor("slides", [2, D], F32, kind="ExternalOutput")
    attn_d = nc.dram_tensor("attn", [P, ntiles], F32, kind="ExternalOutput")

    mult = mybir.AluOpType.mult
    add = mybir.AluOpType.add
    Tanh = mybir.ActivationFunctionType.Tanh
    Exp = mybir.ActivationFunctionType.Exp

    with tile.TileContext(nc) as tc:
        with (
            tc.tile_pool(name="consts", bufs=1) as consts,
            tc.tile_pool(name="xn_pool", bufs=6) as xn_pool,
            tc.tile_pool(name="xt_pool", bufs=4) as xt_pool,
            tc.tile_pool(name="act_pool", bufs=4) as act_pool,
            tc.tile_pool(name="small", bufs=8) as small,
            tc.tile_pool(name="vu_psum", bufs=4, space="PSUM") as vu_psum,
            tc.tile_pool(name="acc_psum", bufs=1, space="PSUM") as acc_psum,
        ):
            if fp8:
                wvwu_s = consts.tile([P, 4, 2, 2 * H], FP8, name="wvwu_s")
                nc.sync.dma_start(out=wvwu_s, in_=wvwu_d.ap())
            else:
                wvwu_s = consts.tile([P, 8, 2 * H], BF16, name="wvwu_s")
                nc.sync.dma_start(
                    out=wvwu_s, in_=wvwu_d.ap().rearrange("c p h -> p c h")
                )
            brow_s = consts.tile([1, 2 * H], BF16, name="brow_s")
            nc.sync.dma_start(out=brow_s, in_=brow_d.ap())
            wtile_s = consts.tile([P, H], BF16, name="wtile_s")
            nc.sync.dma_start(out=wtile_s, in_=wtile_d.ap())
            sel_s = consts.tile([P, ntiles, 2], F32, name="sel_s")
            nc.sync.dma_start(out=sel_s, in_=sel_d.ap())
            id2_s = consts.tile([2, 2], F32, name="id2_s")
            nc.sync.dma_start(out=id2_s, in_=id2_d.ap())

            ones_row = consts.tile([1, P], BF16, name="ones_row")
            nc.vector.memset(ones_row, 1.0)
            ones_col_f = consts.tile([P, 1], F32, name="ones_col_f")
            nc.vector.memset(ones_col_f, 1.0)
            ones_row_f = consts.tile([1, P], F32, name="ones_row_f")
            nc.vector.memset(ones_row_f, 1.0)
            s_acc = consts.tile([P, 2], F32, name="s_acc")
            nc.vector.memset(s_acc, 0.0)

            ubuf = consts.tile([P, ntiles], F32, name="ubuf")
            if mode == "dma":
                nc.vector.memset(ubuf, 1.0)
            if mode == "compute":
                xn_fix = consts.tile([P, D], BF16, name="xn_fix")
                nc.sync.dma_start(out=xn_fix, in_=xn_d.ap()[0:P, :])
                if fp8:
                    xt_fix = consts.tile([P, 4, 2, P], FP8, name="xt_fix")
                else:
                    xt_fix = consts.tile([P, 8, P], BF16, name="xt_fix")
                nc.sync.dma_start(out=xt_fix, in_=xt_d.ap()[0])

            w_ps0 = acc_psum.tile([2, 512], F32, name="w_ps0")
            w_ps1 = acc_psum.tile([2, 512], F32, name="w_ps1")

            if mode == "pe":
                c_fix = consts.tile([P, 2], BF16, name="c_fix")
                nc.vector.memset(c_fix, 0.001)
                nc.vector.memset(ubuf, 1.0)

            # Three-stage software pipeline (A: load+VU matmul+tanh,
            # B: DVE chain+exp, C: coeff+weighted matmuls) so each engine
            # interleaves work from different tiles instead of stalling on
            # the per-tile serial dependency chain.
            state = {}

            def stage_a(t):
                if mode == "compute":
                    xn_t, xt_t = xn_fix, xt_fix
                else:
                    xn_t = xn_pool.tile([P, D], BF16, name="xn_t")
                    nc.sync.dma_start(
                        out=xn_t, in_=xn_d.ap()[t * P : (t + 1) * P, :]
                    )
                    if fp8:
                        xt_t = xt_pool.tile([P, 4, 2, P], FP8, name="xt_t")
                    else:
                        xt_t = xt_pool.tile([P, 8, P], BF16, name="xt_t")
                    nc.sync.dma_start(out=xt_t, in_=xt_d.ap()[t])
                if mode == "dma":
                    return
                vu = vu_psum.tile([P, 2 * H], F32, name="vu", tag="vu")
                # PSUM init with broadcast bias row, then accumulate x @ W.
                nc.tensor.matmul(vu, ones_row, brow_s, start=True, stop=False)
                if fp8:
                    pm = (
                        mybir.MatmulPerfMode.DoubleRowSwInterleave
                        if MM_MODE == "fp8sw"
                        else mybir.MatmulPerfMode.DoubleRow
                    )
                    for c in range(4):
                        nc.tensor.matmul(
                            vu, xt_t[:, c, :, :], wvwu_s[:, c, :, :],
                            start=False, stop=(c == 3),
                            perf_mode=pm,
                        )
                else:
                    for c in range(8):
                        nc.tensor.matmul(
                            vu, xt_t[:, c, :], wvwu_s[:, c, :],
                            start=False, stop=(c == 7),
                        )
                if mode == "pe":
                    state[t] = (xn_t, None)
                    return
                # t12 = tanh([V | U/2 + b/2]) — Wu, bu are pre-halved on host;
                # the fp8 pre-scales are divided out here for free.
                t12 = act_pool.tile([P, 2 * H], BF16, name="t12")
                inv_s = 1.0 / (SW_SCALE * SX_SCALE) if fp8 else 1.0
                nc.scalar.activation(t12, vu, Tanh, scale=inv_s)
                state[t] = (xn_t, t12)

            def stage_b(t):
                if mode in ("dma", "pe"):
                    return
                xn_t, t12 = state[t]
                t1 = t12[:, 0:H]
                t2 = t12[:, H : 2 * H]
                # A = sum_h (w_h/2) * t1 * (1 + t2)
                g = act_pool.tile([P, H], BF16, name="g")
                nc.vector.tensor_mul(g, t1, t2)
                g2 = act_pool.tile([P, H], BF16, name="g2")
                nc.vector.tensor_add(g2, t1, g)
                gw = act_pool.tile([P, H], BF16, name="gw")
                nc.vector.tensor_mul(gw, g2, wtile_s)
                a_log = small.tile([P, 1], F32, name="a_log")
                nc.vector.reduce_sum(a_log, gw, axis=mybir.AxisListType.X)
                nc.scalar.activation(ubuf[:, t : t + 1], a_log, Exp)

            def stage_c(t, first, last):
                if mode == "dma":
                    return
                xn_t, _ = state.pop(t)
                if mode == "pe":
                    c_t = c_fix
                else:
                    c_t = small.tile([P, 2], BF16, name="c_t")
                    nc.vector.tensor_scalar_mul(
                        c_t, sel_s[:, t, :], ubuf[:, t : t + 1]
                    )
                nc.tensor.matmul(
                    w_ps0, c_t, xn_t[:, 0:512],
                    start=first, stop=last, skip_group_check=True,
                )
                nc.tensor.matmul(
                    w_ps1, c_t, xn_t[:, 512:1024],
                    start=first, stop=last, skip_group_check=True,
                )
                if mode != "pe":
                    nc.vector.tensor_add(s_acc, s_acc, c_t)
                elif last:
                    nc.vector.tensor_add(s_acc, s_acc, c_t)

            LAG = 2
            for rep in range(reps):
                for step in range(ntiles + 2 * LAG):
                    ta = step
                    tb = step - LAG
                    tc_ = step - 2 * LAG
                    if ta < ntiles:
                        stage_a(ta)
                    if 0 <= tb < ntiles:
                        stage_b(tb)
                    if 0 <= tc_ < ntiles:
                        stage_c(
                            tc_,
                            first=(rep == 0 and tc_ == 0),
                            last=(rep == reps - 1 and tc_ == ntiles - 1),
                        )

            # ---- epilogue ----
            if mode == "dma":
                slides_z = consts.tile([2, D], F32, name="slides_z")
                nc.vector.memset(slides_z, 0.0)
                nc.sync.dma_start(out=slides_d.ap(), in_=slides_z)
                nc.sync.dma_start(out=attn_d.ap(), in_=ubuf)
            else:
                _epilogue(nc, consts, small, vu_psum, sel_s, ones_row_f, id2_s,
                          ubuf, w_ps0, w_ps1, s_acc, ones_col_f,
                          slides_d, attn_d, ntiles)

    _split_multi_waits(nc)
    return nc


def _epilogue(nc, consts, small, vu_psum, sel_s, ones_row_f, id2_s, ubuf,
              w_ps0, w_ps1, s_acc, ones_col_f, slides_d, attn_d, ntiles):
            s_ps = vu_psum.tile([2, 1], F32, name="s_ps", tag="vu")
            nc.tensor.matmul(s_ps, s_acc, ones_col_f, start=True, stop=True)
            invs = small.tile([2, 1], F32, name="invs")
            nc.vector.reciprocal(invs, s_ps)

            slides_s = consts.tile([2, D], F32, name="slides_s")
            nc.vector.tensor_scalar_mul(slides_s[:, 0:512], w_ps0, invs)
            nc.vector.tensor_scalar_mul(slides_s[:, 512:1024], w_ps1, invs)
            nc.sync.dma_start(out=slides_d.ap(), in_=slides_s)

            # Broadcast 1/S to all 128 partitions: [2,1] -T-> [1,2] -ones-> [128,2]
            invt_ps = vu_psum.tile([1, 2], F32, name="invt_ps", tag="vu")
            nc.tensor.matmul(invt_ps, invs, id2_s, start=True, stop=True)
            invt_s = small.tile([1, 2], F32, name="invt_s")
            nc.vector.tensor_copy(invt_s, invt_ps)
            bc_ps = vu_psum.tile([P, 2], F32, name="bc_ps", tag="vu")
            nc.tensor.matmul(bc_ps, ones_row_f, invt_s, start=True, stop=True)
            invb = small.tile([P, 2], F32, name="invb")
            nc.vector.tensor_copy(invb, bc_ps)

            ra = consts.tile([P, ntiles], F32, name="ra")
            nc.vector.tensor_scalar_mul(ra, sel_s[:, :, 0], invb[:, 0:1])
            rb = consts.tile([P, ntiles], F32, name="rb")
            nc.vector.tensor_scalar_mul(rb, sel_s[:, :, 1], invb[:, 1:2])
            rs = consts.tile([P, ntiles], F32, name="rs")
            nc.vector.tensor_add(rs, ra, rb)
            attn_s = consts.tile([P, ntiles], F32, name="attn_s")
            nc.vector.tensor_mul(attn_s, ubuf, rs)
            nc.sync.dma_start(out=attn_d.ap(), in_=attn_s)


def prepare(inputs):
    """Build the Bass graph and per-core input maps from full inputs."""
    x = np.ascontiguousarray(np.asarray(inputs["x"], dtype=np.float32))
    batch = np.asarray(inputs["batch"]).astype(np.int64)
    Wv = np.asarray(inputs["Wv"], dtype=np.float32)
    bv = np.asarray(inputs["bv"], dtype=np.float32)
    Wu = np.asarray(inputs["Wu"], dtype=np.float32)
    bu = np.asarray(inputs["bu"], dtype=np.float32)
    w = np.asarray(inputs["w"], dtype=np.float32)

    n_total = x.shape[0]
    assert x.shape[1] == D and Wv.shape == (D, H) and w.shape == (H, 1)

    bounds = np.searchsorted(batch, np.arange(B + 1))
    sizes = [int(bounds[2 * k + 2] - bounds[2 * k]) for k in range(N_CORES)]
    ntiles = max(1, -(-max(sizes) // P))
    n_pad = ntiles * P

    nc = _build_nc(ntiles)

    fp8np = mybir.dt.np(FP8)
    # Replicated small tensors. Wu/bu are pre-halved so sigmoid(U) =
    # 0.5*(1 + tanh(U/2)) needs only a plain tanh on the matmul output.
    w_cat = np.concatenate([Wv, 0.5 * Wu], axis=1)  # (D, 512)
    brow_f = np.concatenate([bv, 0.5 * bu]).reshape(1, 2 * H)
    if USE_FP8:
        # wvwu[ki, c, ko, h] = w_cat[c*256 + ko*128 + ki, h], scaled out of
        # e4m3's subnormal range; the ACT tanh divides the scale back out.
        wvwu = np.ascontiguousarray(
            (SW_SCALE * w_cat).reshape(4, 2, P, 2 * H).transpose(2, 0, 1, 3)
        ).astype(fp8np)
        brow_f = brow_f * (SW_SCALE * SX_SCALE)
    else:
        wvwu = w_cat.reshape(8, P, 2 * H).astype(ml_dtypes.bfloat16)
    brow = brow_f.astype(ml_dtypes.bfloat16)
    wtile = np.tile((0.5 * w.reshape(1, H)).astype(ml_dtypes.bfloat16), (P, 1))
    id2 = np.eye(2, dtype=np.float32)

    in_maps = []
    for k in range(N_CORES):
        lo, hi = int(bounds[2 * k]), int(bounds[2 * k + 2])
        nk = hi - lo
        xb = np.zeros((n_pad, D), dtype=ml_dtypes.bfloat16)
        xb[:nk] = x[lo:hi]
        if USE_FP8:
            x8 = np.zeros((n_pad, D), dtype=fp8np)
            x8[:nk] = SX_SCALE * x[lo:hi]
            y = x8.reshape(ntiles, P, 4, 2, P)  # [t, m, c, ko, ki]
            if MM_MODE == "fp8sw":
                # SW-interleaved stationary: per partition ki, the 256
                # weights are [A127 B127 A126 B126 ... A0 B0] where
                # A_m = x[m, ko=0], B_m = x[m, ko=1], columns reversed.
                z = y.transpose(0, 4, 2, 1, 3)  # [t, ki, c, m, ko]
                xt = np.ascontiguousarray(z[:, :, :, ::-1, :]).reshape(
                    ntiles, P, 4, 2, P
                )
            else:
                # HW DoubleRow: xt[t, ki, c, ko, n] = x[t*P+n, c*256+ko*128+ki]
                xt = np.ascontiguousarray(y.transpose(0, 4, 2, 3, 1))
        else:
            # xt[t, p, c, n] = x[t*128 + n, c*128 + p]
            xt = np.ascontiguousarray(
                xb.reshape(ntiles, P, 8, P).transpose(0, 3, 2, 1)
            )
        loc = np.full(n_pad, -1, dtype=np.int64)
        loc[:nk] = batch[lo:hi] - 2 * k
        sel = np.stack([(loc == 0), (loc == 1)], axis=-1).astype(np.float32)
        sel = np.ascontiguousarray(sel.reshape(ntiles, P, 2).transpose(1, 0, 2))
        in_maps.append(
            {
                "xn": xb,
                "xt": xt,
                "sel": sel,
                "wvwu": wvwu,
                "brow": brow,
                "wtile": wtile,
                "id2": id2,
            }
        )

    return nc, in_maps, bounds, n_total


def assemble(results, bounds, n_total):
    slide_features = np.concatenate(
        [results[k]["slides"] for k in range(N_CORES)], axis=0
    ).astype(np.float32)
    attn = np.empty((n_total, 1), dtype=np.float32)
    for k in range(N_CORES):
        lo, hi = int(bounds[2 * k]), int(bounds[2 * k + 2])
        nk = hi - lo
        a = results[k]["attn"]  # [P, ntiles]
        attn[lo:hi, 0] = np.ascontiguousarray(a.T).reshape(-1)[:nk]
    return slide_features, attn


def kernel(**inputs):
    global LAST_RESULT
    nc, in_maps, bounds, n_total = prepare(inputs)
    trace = bool(int(os.environ.get("ABMIL_TRACE", "0")))
    LAST_RESULT = run_bass_kernel_spmd(
        nc, in_maps, core_ids=list(range(N_CORES)), trace=trace
    )
    return assemble(LAST_RESULT.results, bounds, n_total)
